# revision 13
# baseline (speedup 1.0000x reference)
"""Expert-parallel top-1 MoE (SwiGLU experts + shared expert) on 8 TRN2 NeuronCores.

Strategy (hardcoded for B=1, T=256, C=1024, H=2048, E=8):
  - Core e holds expert e's weights (host pre-transposed, bf16) plus a
    1/8 slice (along H) of the shared expert.
  - Every core computes router logits in fp32 (exact argmax), builds a
    token->slot permutation matrix for its own expert on-device, gathers
    its tokens with a matmul, runs the SwiGLU FFN on <=128 token slots in
    bf16 (fp32 accumulation), and scatters results back with a matmul,
    accumulating its shared-expert partial in the same PSUM banks.
  - Each core writes a disjoint-support partial of the full [C, T] output;
    the host sums the 8 partials and transposes back to [1, T, C].

Schedule notes:
  - All small inputs ship as two packed buffers (one fp32, one bf16) so the
    routing path lands in ~2 DMAs instead of ~10.
  - Expert weights stream as ~1MB chunks, alternating the two HWDGE rings
    (sync + scalar), ordered by FFN consumption (H-half 0 first, down last).
  - A short burst of dummy matmuls warms the PE clock (HAM) while DMA runs.
"""

import sys

if "/opt/trn_rl_repo" not in sys.path:
    sys.path.insert(0, "/opt/trn_rl_repo")

import ml_dtypes
import numpy as np

B, T, C, H, E = 1, 256, 1024, 2048, 8
HS = H // 8        # shared-expert hidden slice per core
CCAP = 128         # per-expert token capacity (binomial mean 32; 128 is >12 sigma)
BF16 = ml_dtypes.bfloat16

# f32 pack layout (per-partition free offsets)
O_XT32, O_ROUT, O_IOTA, O_EID = 0, 2048, 2112, 2240
F32LEN = 2241
# bf16 pack layout
O_XB, O_XTB, O_WUP, O_WGATE, O_WDOWN, O_TRIU, O_IDB = (
    0, 2048, 4096, 6144, 8192, 10240, 10752)
BFLEN = 10880

N_WARM = 35

_CACHE = {}


def _build_program():
    import concourse.tile as tile
    from concourse import bacc, mybir

    f32 = mybir.dt.float32
    bf16 = mybir.dt.bfloat16
    u32 = mybir.dt.uint32
    ALU = mybir.AluOpType
    ACT = mybir.ActivationFunctionType

    nc = bacc.Bacc("TRN2", target_bir_lowering=False, debug=False, num_devices=8)

    f32pack = nc.dram_tensor("f32pack", [128, F32LEN], f32, kind="ExternalInput").ap()
    bfpack = nc.dram_tensor("bfpack", [128, BFLEN], bf16, kind="ExternalInput").ap()
    upT = nc.dram_tensor("upT", [C, H], bf16, kind="ExternalInput").ap()
    gateT = nc.dram_tensor("gateT", [C, H], bf16, kind="ExternalInput").ap()
    downT = nc.dram_tensor("downT", [H, C], bf16, kind="ExternalInput").ap()
    outT = nc.dram_tensor("outT", [C, T], f32, kind="ExternalOutput").ap()

    upTv = upT.rearrange("(a p) h -> p a h", p=128)        # [128, 8, 2048]
    gateTv = gateT.rearrange("(a p) h -> p a h", p=128)
    downTv = downT.rearrange("(a p) c -> p a c", p=128)    # [128, 16, 1024]
    outTv = outT.rearrange("(a p) t -> p a t", p=128)      # [128, 8, 256]

    with tile.TileContext(nc) as tc:
        with (
            tc.tile_pool(name="consts", bufs=1) as consts,
            tc.tile_pool(name="wts", bufs=1) as wts,
            tc.tile_pool(name="tmp", bufs=2) as tmp,
        ):
            # ---- packed small inputs: one DMA per ring ----
            fp_sb = consts.tile([128, F32LEN], f32, tag="fp")
            nc.sync.dma_start(fp_sb[:], f32pack[:])
            bp_sb = consts.tile([128, BFLEN], bf16, tag="bp")
            nc.scalar.dma_start(bp_sb[:], bfpack[:])

            # slice helpers into the packs
            def xT32s(k, tt):           # fp32 x^T tile [128, 128] (lhsT for logits)
                o = O_XT32 + k * 256 + tt * 128
                return fp_sb[:, o:o + 128]

            def routs(k):               # routerT [128, 8]
                o = O_ROUT + k * 8
                return fp_sb[:, o:o + 8]

            iota_s = fp_sb[:, O_IOTA:O_IOTA + CCAP]
            eid_s = fp_sb[:, O_EID:O_EID + 1]

            def xbs(tt, m):             # x bf16 [128(t), 128(c)]
                o = O_XB + tt * 1024 + m * 128
                return bp_sb[:, o:o + 128]

            def xTbs(k):                # x^T bf16 [128, 256]
                o = O_XTB + k * 256
                return bp_sb[:, o:o + 256]

            def wups(k, st):
                o = O_WUP + k * 256 + st * 128
                return bp_sb[:, o:o + 128]

            def wgates(k, st):
                o = O_WGATE + k * 256 + st * 128
                return bp_sb[:, o:o + 128]

            def wdowns(st, m):
                o = O_WDOWN + st * 1024 + m * 128
                return bp_sb[:, o:o + 128]

            def trius(kt, mt):
                o = O_TRIU + kt * 256 + mt * 128
                return bp_sb[:, o:o + 128]

            idb_s = bp_sb[:, O_IDB:O_IDB + 128]

            # ---- expert weight chunks, ~1MB, consumption order ----
            # up/gate: [128, 4 K-tiles, H-half]; down: [128, 4 K-tiles, C]
            upc = [wts.tile([128, 4, 1024], bf16, tag=f"upc{i}", name=f"upc{i}")
                   for i in range(4)]
            gatec = [wts.tile([128, 4, 1024], bf16, tag=f"gac{i}", name=f"gac{i}")
                     for i in range(4)]
            downc = [wts.tile([128, 4, C], bf16, tag=f"doc{i}", name=f"doc{i}")
                     for i in range(4)]
            # up/gate stream on the sync ring in consumption order; down + bfpack
            # ride the scalar ring (few issues, all early, so the ACT engine's
            # queue never blocks a later silu on DMA ring credits).
            for i in range(4):
                hh, kg = i // 2, i % 2
                hsl = slice(hh * 1024, (hh + 1) * 1024)
                nc.sync.dma_start(upc[i][:], upTv[:, kg * 4:(kg + 1) * 4, hsl])
                nc.sync.dma_start(gatec[i][:], gateTv[:, kg * 4:(kg + 1) * 4, hsl])
            for q in range(4):
                nc.scalar.dma_start(downc[q][:], downTv[:, q * 4:(q + 1) * 4, :])

            # ---- PE warmup: dummy matmuls while DMA streams ----
            warm_sb = consts.tile([128, 256], bf16, tag="warm")
            nc.vector.memset(warm_sb[:], 0.0)
            # also pre-load the ACT engine's Silu table during the idle preamble
            warm_act = tmp.tile([128, 8], bf16, tag="warm_act")
            nc.scalar.activation(warm_act[:], warm_sb[:, 0:8], ACT.Silu)
            with tc.tile_pool(name="psW", bufs=1, space="PSUM") as psW:
                w_ps = psW.tile([128, 128], f32, tag="w")
                for _ in range(N_WARM):
                    nc.tensor.matmul(
                        w_ps[:], lhsT=warm_sb[:, 0:128], rhs=warm_sb[:, 128:256],
                        start=True, stop=True,
                    )

            # ---- routing (fp32 logits) + shared expert on PE ----
            mask_sb = consts.tile([128, 2, 1], f32, tag="mask")
            maskb_sb = consts.tile([128, 2, 1], bf16, tag="maskb")
            hsT_sb = consts.tile([128, 2, T], bf16, tag="hsT")
            possel_sb = consts.tile([128, 2, 1], f32, tag="possel")
            with tc.tile_pool(name="psA", bufs=2, space="PSUM") as psA:
                for tt in range(2):
                    lg_ps = psA.tile([128, E], f32, tag="lg")
                    for k in range(8):
                        nc.tensor.matmul(
                            lg_ps[:], lhsT=xT32s(k, tt), rhs=routs(k),
                            start=(k == 0), stop=(k == 7),
                        )
                    lg_sb = tmp.tile([128, E], f32, tag="lg_sb")
                    nc.vector.tensor_copy(lg_sb[:], lg_ps[:])
                    mx = tmp.tile([128, 8], f32, tag="mx")
                    nc.vector.max(mx[:], lg_sb[:])
                    mi = tmp.tile([128, 8], u32, tag="mi")
                    nc.vector.max_index(mi[:], mx[:], lg_sb[:])
                    idxf = tmp.tile([128, 1], f32, tag="idxf")
                    nc.vector.tensor_copy(idxf[:], mi[:, 0:1])
                    nc.vector.tensor_tensor(
                        mask_sb[:, tt, :], idxf[:], eid_s, op=ALU.is_equal
                    )
                    nc.vector.tensor_copy(maskb_sb[:, tt, :], mask_sb[:, tt, :])

                def shared_half(st):
                    us_ps = psA.tile([128, T], f32, tag="us")
                    gs_ps = psA.tile([128, T], f32, tag="gs")
                    for k in range(8):
                        nc.tensor.matmul(
                            us_ps[:], lhsT=wups(k, st), rhs=xTbs(k),
                            start=(k == 0), stop=(k == 7),
                        )
                        nc.tensor.matmul(
                            gs_ps[:], lhsT=wgates(k, st), rhs=xTbs(k),
                            start=(k == 0), stop=(k == 7),
                        )
                    sils = tmp.tile([128, T], bf16, tag="sils")
                    nc.scalar.activation(sils[:], gs_ps[:], ACT.Silu)
                    nc.vector.tensor_tensor(
                        hsT_sb[:, st, :], sils[:], us_ps[:], op=ALU.mult
                    )

                shared_half(0)

                # positions via triu matmul: cumsum(mask)[t] - 1, unrouted -> -2
                for mt in range(2):
                    pos_ps = psA.tile([128, 1], f32, tag="lg")
                    for kt in range(2):
                        nc.tensor.matmul(
                            pos_ps[:], lhsT=trius(kt, mt), rhs=maskb_sb[:, kt, :],
                            start=(kt == 0), stop=(kt == 1),
                        )
                    pos1 = tmp.tile([128, 1], f32, tag="pos1")
                    nc.vector.tensor_scalar(
                        pos1[:], pos_ps[:], 1.0, None, op0=ALU.add
                    )
                    posm = tmp.tile([128, 1], f32, tag="posm")
                    nc.vector.tensor_tensor(
                        posm[:], pos1[:], mask_sb[:, mt, :], op=ALU.mult
                    )
                    nc.vector.tensor_scalar(
                        possel_sb[:, mt, :], posm[:], 2.0, None, op0=ALU.subtract
                    )

                shared_half(1)

            # ---- permutation matrices + gather ----
            permT_sb = consts.tile([128, 2, CCAP], bf16, tag="permT")
            perm_sb = consts.tile([128, 2 * 128], bf16, tag="perm")
            gx_sb = consts.tile([128, 8, CCAP], bf16, tag="gx")
            with tc.tile_pool(name="psB", bufs=2, space="PSUM") as psB:
                for tt in range(2):
                    nc.vector.tensor_scalar(
                        permT_sb[:, tt, :], iota_s, possel_sb[:, tt, :], None,
                        op0=ALU.is_equal,
                    )
                for tt in range(2):
                    pt_ps = psB.tile([128, 128], bf16, tag="pt")
                    nc.tensor.transpose(pt_ps[:], permT_sb[:, tt, :], idb_s)
                    nc.vector.tensor_copy(
                        perm_sb[:, tt * 128:(tt + 1) * 128], pt_ps[:]
                    )
                for m in range(8):
                    g_ps = psB.tile([128, CCAP], f32, tag="gps")
                    for tt in range(2):
                        nc.tensor.matmul(
                            g_ps[:], lhsT=xbs(tt, m), rhs=permT_sb[:, tt, :],
                            start=(tt == 0), stop=(tt == 1),
                        )
                    nc.vector.tensor_copy(gx_sb[:, m, :], g_ps[:])

            # ---- routed FFN: tokens stationary, weights streaming ----
            hT_sb = consts.tile([128, 16, CCAP], bf16, tag="hT")
            y_sb = consts.tile([128, C], bf16, tag="y")
            with tc.tile_pool(name="psy", bufs=1, space="PSUM") as psy:
                y_ps = psy.tile([128, C], f32, tag="yps")
                with (
                    tc.tile_pool(name="psu", bufs=1, space="PSUM") as psu,
                    tc.tile_pool(name="pst", bufs=2, space="PSUM") as pst,
                ):
                    for hh in range(2):
                        u_ps = psu.tile([128, 1024], f32, tag="u")
                        g_ps = psu.tile([128, 1024], f32, tag="g")
                        for cc in range(2):
                            dst = slice(cc * 512, (cc + 1) * 512)
                            for k in range(8):
                                ch = upc[hh * 2 + k // 4]
                                gh = gatec[hh * 2 + k // 4]
                                wsl = slice(cc * 512, (cc + 1) * 512)
                                nc.tensor.matmul(
                                    u_ps[:, dst], lhsT=gx_sb[:, k, :],
                                    rhs=ch[:, k % 4, wsl],
                                    start=(k == 0), stop=(k == 7),
                                )
                                nc.tensor.matmul(
                                    g_ps[:, dst], lhsT=gx_sb[:, k, :],
                                    rhs=gh[:, k % 4, wsl],
                                    start=(k == 0), stop=(k == 7),
                                )
                            sil = tmp.tile([128, 512], bf16, tag="sil")
                            nc.scalar.activation(sil[:], g_ps[:, dst], ACT.Silu)
                            h_sb = tmp.tile([128, 512], bf16, tag="h")
                            nc.vector.tensor_tensor(
                                h_sb[:], sil[:], u_ps[:, dst], op=ALU.mult
                            )
                            for j4 in range(4):
                                t_ps = pst.tile([128, 128], bf16, tag="tr")
                                nc.tensor.transpose(
                                    t_ps[:], h_sb[:, j4 * 128:(j4 + 1) * 128], idb_s
                                )
                                nc.vector.tensor_copy(
                                    hT_sb[:, hh * 8 + cc * 4 + j4, :], t_ps[:]
                                )

                # ---- down (C-half groups) + fused scatter/shared-down/out ----
                with tc.tile_pool(name="pso", bufs=2, space="PSUM") as pso:
                    for ccc in range(2):
                        dst = slice(ccc * 512, (ccc + 1) * 512)
                        for jj in range(16):
                            nc.tensor.matmul(
                                y_ps[:, dst], lhsT=hT_sb[:, jj, :],
                                rhs=downc[jj // 4][:, jj % 4, dst],
                                start=(jj == 0), stop=(jj == 15),
                            )
                        nc.vector.tensor_copy(y_sb[:, dst], y_ps[:, dst])
                        for half in range(2):
                            o_sb = tmp.tile([128, 2 * T], f32, tag="o_sb")
                            for mm in range(2):
                                m = ccc * 4 + half * 2 + mm
                                o_ps = pso.tile([128, T], f32, tag="o")
                                nc.tensor.matmul(
                                    o_ps[:], lhsT=y_sb[:, m * 128:(m + 1) * 128],
                                    rhs=perm_sb[:], start=True, stop=False,
                                )
                                for st in range(2):
                                    nc.tensor.matmul(
                                        o_ps[:], lhsT=wdowns(st, m),
                                        rhs=hsT_sb[:, st, :],
                                        start=False, stop=(st == 1),
                                    )
                                nc.vector.tensor_copy(
                                    o_sb[:, mm * T:(mm + 1) * T], o_ps[:]
                                )
                            nc.sync.dma_start(
                                outTv[:, ccc * 4 + half * 2:ccc * 4 + half * 2 + 2, :],
                                o_sb[:].rearrange("p (a t) -> p a t", t=T),
                            )

    nc.compile()
    return nc


def _get_program():
    if "nc" not in _CACHE:
        _CACHE["nc"] = _build_program()
    return _CACHE["nc"]


def _pack_inputs(x, up, gate, down, router, w_up_s, w_gate_s, w_down_s):
    f32 = np.float32
    x2 = np.ascontiguousarray(x.reshape(T, C)).astype(f32, copy=False)
    xT = np.ascontiguousarray(x2.T)

    def fold_cols(a):
        # [R, F] with R = n*128 -> [128, n*F] grouping k-tiles along free dim
        n = a.shape[0] // 128
        return a.reshape(n, 128, a.shape[1]).transpose(1, 0, 2).reshape(128, -1)

    fp = np.zeros((128, F32LEN), f32)
    fp[:, O_XT32:O_XT32 + 2048] = fold_cols(xT)
    fp[:, O_ROUT:O_ROUT + 64] = fold_cols(
        np.ascontiguousarray(router.astype(f32, copy=False).T))
    fp[:, O_IOTA:O_IOTA + CCAP] = np.arange(CCAP, dtype=f32)[None, :]

    bp = np.zeros((128, BFLEN), BF16)
    bp[:, O_XB:O_XB + 2048] = fold_cols(x2).astype(BF16)
    bp[:, O_XTB:O_XTB + 2048] = fold_cols(xT).astype(BF16)
    bp[:, O_TRIU:O_TRIU + 512] = fold_cols(np.triu(np.ones((T, T), f32))).astype(BF16)
    bp[:, O_IDB:O_IDB + 128] = np.eye(128, dtype=f32).astype(BF16)

    in_maps = []
    for e in range(E):
        sl = slice(e * HS, (e + 1) * HS)
        fpe = fp.copy()
        fpe[:, O_EID] = float(e)
        bpe = bp.copy()
        bpe[:, O_WUP:O_WUP + 2048] = fold_cols(
            np.ascontiguousarray(w_up_s[sl, :].astype(f32, copy=False).T)).astype(BF16)
        bpe[:, O_WGATE:O_WGATE + 2048] = fold_cols(
            np.ascontiguousarray(w_gate_s[sl, :].astype(f32, copy=False).T)).astype(BF16)
        bpe[:, O_WDOWN:O_WDOWN + 2048] = fold_cols(
            np.ascontiguousarray(w_down_s[:, sl].astype(f32, copy=False).T)).astype(BF16)
        m = {
            "f32pack": fpe,
            "bfpack": bpe,
            "upT": np.ascontiguousarray(up[e].astype(f32, copy=False).T.astype(BF16)),
            "gateT": np.ascontiguousarray(gate[e].astype(f32, copy=False).T.astype(BF16)),
            "downT": np.ascontiguousarray(down[e].astype(f32, copy=False).T.astype(BF16)),
        }
        in_maps.append(m)
    return in_maps


_make_in_maps = _pack_inputs


def run_spmd(in_maps, **kwargs):
    from concourse.bass_utils import run_bass_kernel_spmd

    nc = _get_program()
    return run_bass_kernel_spmd(nc, in_maps, core_ids=list(range(8)), **kwargs)


def kernel(x, up, gate, down, router, w_up_s, w_gate_s, w_down_s):
    in_maps = _pack_inputs(
        np.asarray(x), np.asarray(up), np.asarray(gate), np.asarray(down),
        np.asarray(router), np.asarray(w_up_s), np.asarray(w_gate_s),
        np.asarray(w_down_s),
    )
    res = run_spmd(in_maps)
    acc = np.zeros((C, T), np.float32)
    for i in range(E):
        acc += res.results[i]["outT"]
    return np.ascontiguousarray(acc.T).reshape(B, T, C).astype(np.float32)


# revision 15
# speedup vs baseline: 1.0481x; 1.0481x over previous
"""Expert-parallel top-1 MoE (SwiGLU experts + shared expert) on 8 TRN2 NeuronCores.

Strategy (hardcoded for B=1, T=256, C=1024, H=2048, E=8):
  - Core e holds expert e's weights (host pre-transposed, bf16) plus a
    1/8 slice (along H) of the shared expert.
  - Every core computes router logits in fp32 (exact argmax), builds a
    token->slot permutation matrix for its own expert on-device, gathers
    its tokens with a matmul, runs the SwiGLU FFN on <=128 token slots in
    bf16 (fp32 accumulation), and scatters results back with a matmul,
    accumulating its shared-expert partial in the same PSUM banks.
  - Each core writes a disjoint-support partial of the full [C, T] output;
    the host sums the 8 partials and transposes back to [1, T, C].

Schedule notes:
  - All small inputs ship as two packed buffers (one fp32, one bf16) so the
    routing path lands in ~2 DMAs instead of ~10.
  - Expert weights stream as ~1MB chunks, alternating the two HWDGE rings
    (sync + scalar), ordered by FFN consumption (H-half 0 first, down last).
  - A short burst of dummy matmuls warms the PE clock (HAM) while DMA runs.
"""

import sys

if "/opt/trn_rl_repo" not in sys.path:
    sys.path.insert(0, "/opt/trn_rl_repo")

import ml_dtypes
import numpy as np

B, T, C, H, E = 1, 256, 1024, 2048, 8
HS = H // 8        # shared-expert hidden slice per core
CCAP = 128         # per-expert token capacity (binomial mean 32; 128 is >12 sigma)
BF16 = ml_dtypes.bfloat16

# f32 pack layout (per-partition free offsets)
O_XT32, O_ROUT, O_IOTA, O_EID = 0, 2048, 2112, 2240
F32LEN = 2241
# bf16 pack layout
O_XB, O_XTB, O_WUP, O_WGATE, O_WDOWN, O_TRIU, O_IDB = (
    0, 2048, 4096, 6144, 8192, 10240, 10752)
BFLEN = 10880

N_WARM = 35

_CACHE = {}


def _build_program():
    import concourse.tile as tile
    from concourse import bacc, mybir

    f32 = mybir.dt.float32
    bf16 = mybir.dt.bfloat16
    u32 = mybir.dt.uint32
    ALU = mybir.AluOpType
    ACT = mybir.ActivationFunctionType

    nc = bacc.Bacc("TRN2", target_bir_lowering=False, debug=False, num_devices=8)

    f32pack = nc.dram_tensor("f32pack", [128, F32LEN], f32, kind="ExternalInput").ap()
    bfpack = nc.dram_tensor("bfpack", [128, BFLEN], bf16, kind="ExternalInput").ap()
    upT = nc.dram_tensor("upT", [C, H], bf16, kind="ExternalInput").ap()
    gateT = nc.dram_tensor("gateT", [C, H], bf16, kind="ExternalInput").ap()
    downT = nc.dram_tensor("downT", [H, C], bf16, kind="ExternalInput").ap()
    outT = nc.dram_tensor("outT", [C, T], f32, kind="ExternalOutput").ap()

    upTv = upT.rearrange("(a p) h -> p a h", p=128)        # [128, 8, 2048]
    gateTv = gateT.rearrange("(a p) h -> p a h", p=128)
    downTv = downT.rearrange("(a p) c -> p a c", p=128)    # [128, 16, 1024]
    outTv = outT.rearrange("(a p) t -> p a t", p=128)      # [128, 8, 256]

    with tile.TileContext(nc) as tc:
        with (
            tc.tile_pool(name="consts", bufs=1) as consts,
            tc.tile_pool(name="wts", bufs=1) as wts,
            tc.tile_pool(name="tmp", bufs=2) as tmp,
        ):
            # ---- packed small inputs ----
            # In-flight DMAs share SDMA bandwidth round-robin regardless of
            # issue order, so enforce strict phases (fp -> bp -> up/gate ->
            # down) with tiny gating copies on the otherwise-idle GpSimd
            # engine: each copy pre-writes one element of the next phase's
            # target tile while reading from the previous phase's tile, which
            # makes the next DMA wait (WAW) for the previous phase to land.
            fp_sb = consts.tile([128, F32LEN], f32, tag="fp")
            nc.sync.dma_start(fp_sb[:], f32pack[:])
            bp_sb = consts.tile([128, BFLEN], bf16, tag="bp")
            nc.gpsimd.tensor_copy(bp_sb[:, 0:1], fp_sb[:, 0:1])
            nc.scalar.dma_start(bp_sb[:], bfpack[:])

            # slice helpers into the packs
            def xT32s(k, tt):           # fp32 x^T tile [128, 128] (lhsT for logits)
                o = O_XT32 + k * 256 + tt * 128
                return fp_sb[:, o:o + 128]

            def routs(k):               # routerT [128, 8]
                o = O_ROUT + k * 8
                return fp_sb[:, o:o + 8]

            iota_s = fp_sb[:, O_IOTA:O_IOTA + CCAP]
            eid_s = fp_sb[:, O_EID:O_EID + 1]

            def xbs(tt, m):             # x bf16 [128(t), 128(c)]
                o = O_XB + tt * 1024 + m * 128
                return bp_sb[:, o:o + 128]

            def xTbs(k):                # x^T bf16 [128, 256]
                o = O_XTB + k * 256
                return bp_sb[:, o:o + 256]

            def wups(k, st):
                o = O_WUP + k * 256 + st * 128
                return bp_sb[:, o:o + 128]

            def wgates(k, st):
                o = O_WGATE + k * 256 + st * 128
                return bp_sb[:, o:o + 128]

            def wdowns(st, m):
                o = O_WDOWN + st * 1024 + m * 128
                return bp_sb[:, o:o + 128]

            def trius(kt, mt):
                o = O_TRIU + kt * 256 + mt * 128
                return bp_sb[:, o:o + 128]

            idb_s = bp_sb[:, O_IDB:O_IDB + 128]

            # ---- expert weight chunks, ~1MB, consumption order ----
            # up/gate: [128, 4 K-tiles, H-half]; down: [128, 4 K-tiles, C]
            upc = [wts.tile([128, 4, 1024], bf16, tag=f"upc{i}", name=f"upc{i}")
                   for i in range(4)]
            gatec = [wts.tile([128, 4, 1024], bf16, tag=f"gac{i}", name=f"gac{i}")
                     for i in range(4)]
            downc = [wts.tile([128, 4, C], bf16, tag=f"doc{i}", name=f"doc{i}")
                     for i in range(4)]
            # All big weights stream on the sync ring only (the scalar/ACT
            # queue must stay clear for silu), gated into phases as above.
            for i in range(4):
                hh, kg = i // 2, i % 2
                hsl = slice(hh * 1024, (hh + 1) * 1024)
                nc.gpsimd.tensor_copy(upc[i][:, 0, 0:1], bp_sb[:, 0:1])
                nc.sync.dma_start(upc[i][:], upTv[:, kg * 4:(kg + 1) * 4, hsl])
                nc.gpsimd.tensor_copy(gatec[i][:, 0, 0:1], bp_sb[:, 0:1])
                nc.sync.dma_start(gatec[i][:], gateTv[:, kg * 4:(kg + 1) * 4, hsl])
            for q in range(4):
                nc.gpsimd.tensor_copy(downc[q][:, 0, 0:1], upc[3][:, 0, 0:1])
                nc.sync.dma_start(downc[q][:], downTv[:, q * 4:(q + 1) * 4, :])

            # ---- PE warmup: dummy matmuls while DMA streams ----
            warm_sb = consts.tile([128, 256], bf16, tag="warm")
            nc.vector.memset(warm_sb[:], 0.0)
            # also pre-load the ACT engine's Silu table during the idle preamble
            warm_act = tmp.tile([128, 8], bf16, tag="warm_act")
            nc.scalar.activation(warm_act[:], warm_sb[:, 0:8], ACT.Silu)
            with tc.tile_pool(name="psW", bufs=1, space="PSUM") as psW:
                w_ps = psW.tile([128, 128], f32, tag="w")
                for _ in range(N_WARM):
                    nc.tensor.matmul(
                        w_ps[:], lhsT=warm_sb[:, 0:128], rhs=warm_sb[:, 128:256],
                        start=True, stop=True,
                    )

            # ---- routing (fp32 logits) + shared expert on PE ----
            mask_sb = consts.tile([128, 2, 1], f32, tag="mask")
            maskb_sb = consts.tile([128, 2, 1], bf16, tag="maskb")
            hsT_sb = consts.tile([128, 2, T], bf16, tag="hsT")
            possel_sb = consts.tile([128, 2, 1], f32, tag="possel")
            with tc.tile_pool(name="psA", bufs=2, space="PSUM") as psA:
                for tt in range(2):
                    lg_ps = psA.tile([128, E], f32, tag="lg")
                    for k in range(8):
                        nc.tensor.matmul(
                            lg_ps[:], lhsT=xT32s(k, tt), rhs=routs(k),
                            start=(k == 0), stop=(k == 7),
                        )
                    lg_sb = tmp.tile([128, E], f32, tag="lg_sb")
                    nc.vector.tensor_copy(lg_sb[:], lg_ps[:])
                    mx = tmp.tile([128, 8], f32, tag="mx")
                    nc.vector.max(mx[:], lg_sb[:])
                    mi = tmp.tile([128, 8], u32, tag="mi")
                    nc.vector.max_index(mi[:], mx[:], lg_sb[:])
                    idxf = tmp.tile([128, 1], f32, tag="idxf")
                    nc.vector.tensor_copy(idxf[:], mi[:, 0:1])
                    nc.vector.tensor_tensor(
                        mask_sb[:, tt, :], idxf[:], eid_s, op=ALU.is_equal
                    )
                    nc.vector.tensor_copy(maskb_sb[:, tt, :], mask_sb[:, tt, :])

                def shared_half(st):
                    us_ps = psA.tile([128, T], f32, tag="us")
                    gs_ps = psA.tile([128, T], f32, tag="gs")
                    for k in range(8):
                        nc.tensor.matmul(
                            us_ps[:], lhsT=wups(k, st), rhs=xTbs(k),
                            start=(k == 0), stop=(k == 7),
                        )
                        nc.tensor.matmul(
                            gs_ps[:], lhsT=wgates(k, st), rhs=xTbs(k),
                            start=(k == 0), stop=(k == 7),
                        )
                    sils = tmp.tile([128, T], bf16, tag="sils")
                    nc.scalar.activation(sils[:], gs_ps[:], ACT.Silu)
                    nc.vector.tensor_tensor(
                        hsT_sb[:, st, :], sils[:], us_ps[:], op=ALU.mult
                    )

                shared_half(0)

                # positions via triu matmul: cumsum(mask)[t] - 1, unrouted -> -2
                for mt in range(2):
                    pos_ps = psA.tile([128, 1], f32, tag="lg")
                    for kt in range(2):
                        nc.tensor.matmul(
                            pos_ps[:], lhsT=trius(kt, mt), rhs=maskb_sb[:, kt, :],
                            start=(kt == 0), stop=(kt == 1),
                        )
                    pos1 = tmp.tile([128, 1], f32, tag="pos1")
                    nc.vector.tensor_scalar(
                        pos1[:], pos_ps[:], 1.0, None, op0=ALU.add
                    )
                    posm = tmp.tile([128, 1], f32, tag="posm")
                    nc.vector.tensor_tensor(
                        posm[:], pos1[:], mask_sb[:, mt, :], op=ALU.mult
                    )
                    nc.vector.tensor_scalar(
                        possel_sb[:, mt, :], posm[:], 2.0, None, op0=ALU.subtract
                    )

                shared_half(1)

            # ---- permutation matrices + gather ----
            permT_sb = consts.tile([128, 2, CCAP], bf16, tag="permT")
            perm_sb = consts.tile([128, 2 * 128], bf16, tag="perm")
            gx_sb = consts.tile([128, 8, CCAP], bf16, tag="gx")
            with tc.tile_pool(name="psB", bufs=2, space="PSUM") as psB:
                for tt in range(2):
                    nc.vector.tensor_scalar(
                        permT_sb[:, tt, :], iota_s, possel_sb[:, tt, :], None,
                        op0=ALU.is_equal,
                    )
                for tt in range(2):
                    pt_ps = psB.tile([128, 128], bf16, tag="pt")
                    nc.tensor.transpose(pt_ps[:], permT_sb[:, tt, :], idb_s)
                    nc.vector.tensor_copy(
                        perm_sb[:, tt * 128:(tt + 1) * 128], pt_ps[:]
                    )
                for m in range(8):
                    g_ps = psB.tile([128, CCAP], f32, tag="gps")
                    for tt in range(2):
                        nc.tensor.matmul(
                            g_ps[:], lhsT=xbs(tt, m), rhs=permT_sb[:, tt, :],
                            start=(tt == 0), stop=(tt == 1),
                        )
                    nc.vector.tensor_copy(gx_sb[:, m, :], g_ps[:])

            # ---- routed FFN: tokens stationary, weights streaming ----
            hT_sb = consts.tile([128, 16, CCAP], bf16, tag="hT")
            y_sb = consts.tile([128, C], bf16, tag="y")
            with tc.tile_pool(name="psy", bufs=1, space="PSUM") as psy:
                y_ps = psy.tile([128, C], f32, tag="yps")
                with (
                    tc.tile_pool(name="psu", bufs=1, space="PSUM") as psu,
                    tc.tile_pool(name="pst", bufs=2, space="PSUM") as pst,
                ):
                    for hh in range(2):
                        u_ps = psu.tile([128, 1024], f32, tag="u")
                        g_ps = psu.tile([128, 1024], f32, tag="g")
                        for cc in range(2):
                            dst = slice(cc * 512, (cc + 1) * 512)
                            for k in range(8):
                                ch = upc[hh * 2 + k // 4]
                                gh = gatec[hh * 2 + k // 4]
                                wsl = slice(cc * 512, (cc + 1) * 512)
                                nc.tensor.matmul(
                                    u_ps[:, dst], lhsT=gx_sb[:, k, :],
                                    rhs=ch[:, k % 4, wsl],
                                    start=(k == 0), stop=(k == 7),
                                )
                                nc.tensor.matmul(
                                    g_ps[:, dst], lhsT=gx_sb[:, k, :],
                                    rhs=gh[:, k % 4, wsl],
                                    start=(k == 0), stop=(k == 7),
                                )
                            sil = tmp.tile([128, 512], bf16, tag="sil")
                            nc.scalar.activation(sil[:], g_ps[:, dst], ACT.Silu)
                            h_sb = tmp.tile([128, 512], bf16, tag="h")
                            nc.vector.tensor_tensor(
                                h_sb[:], sil[:], u_ps[:, dst], op=ALU.mult
                            )
                            for j4 in range(4):
                                t_ps = pst.tile([128, 128], bf16, tag="tr")
                                nc.tensor.transpose(
                                    t_ps[:], h_sb[:, j4 * 128:(j4 + 1) * 128], idb_s
                                )
                                nc.vector.tensor_copy(
                                    hT_sb[:, hh * 8 + cc * 4 + j4, :], t_ps[:]
                                )

                # ---- down (C-half groups) + fused scatter/shared-down/out ----
                with tc.tile_pool(name="pso", bufs=2, space="PSUM") as pso:
                    for ccc in range(2):
                        dst = slice(ccc * 512, (ccc + 1) * 512)
                        for jj in range(16):
                            nc.tensor.matmul(
                                y_ps[:, dst], lhsT=hT_sb[:, jj, :],
                                rhs=downc[jj // 4][:, jj % 4, dst],
                                start=(jj == 0), stop=(jj == 15),
                            )
                        nc.vector.tensor_copy(y_sb[:, dst], y_ps[:, dst])
                        for half in range(2):
                            o_sb = tmp.tile([128, 2 * T], f32, tag="o_sb")
                            for mm in range(2):
                                m = ccc * 4 + half * 2 + mm
                                o_ps = pso.tile([128, T], f32, tag="o")
                                nc.tensor.matmul(
                                    o_ps[:], lhsT=y_sb[:, m * 128:(m + 1) * 128],
                                    rhs=perm_sb[:], start=True, stop=False,
                                )
                                for st in range(2):
                                    nc.tensor.matmul(
                                        o_ps[:], lhsT=wdowns(st, m),
                                        rhs=hsT_sb[:, st, :],
                                        start=False, stop=(st == 1),
                                    )
                                nc.vector.tensor_copy(
                                    o_sb[:, mm * T:(mm + 1) * T], o_ps[:]
                                )
                            nc.sync.dma_start(
                                outTv[:, ccc * 4 + half * 2:ccc * 4 + half * 2 + 2, :],
                                o_sb[:].rearrange("p (a t) -> p a t", t=T),
                            )

    nc.compile()
    return nc


def _get_program():
    if "nc" not in _CACHE:
        _CACHE["nc"] = _build_program()
    return _CACHE["nc"]


def _pack_inputs(x, up, gate, down, router, w_up_s, w_gate_s, w_down_s):
    f32 = np.float32
    x2 = np.ascontiguousarray(x.reshape(T, C)).astype(f32, copy=False)
    xT = np.ascontiguousarray(x2.T)

    def fold_cols(a):
        # [R, F] with R = n*128 -> [128, n*F] grouping k-tiles along free dim
        n = a.shape[0] // 128
        return a.reshape(n, 128, a.shape[1]).transpose(1, 0, 2).reshape(128, -1)

    fp = np.zeros((128, F32LEN), f32)
    fp[:, O_XT32:O_XT32 + 2048] = fold_cols(xT)
    fp[:, O_ROUT:O_ROUT + 64] = fold_cols(
        np.ascontiguousarray(router.astype(f32, copy=False).T))
    fp[:, O_IOTA:O_IOTA + CCAP] = np.arange(CCAP, dtype=f32)[None, :]

    bp = np.zeros((128, BFLEN), BF16)
    bp[:, O_XB:O_XB + 2048] = fold_cols(x2).astype(BF16)
    bp[:, O_XTB:O_XTB + 2048] = fold_cols(xT).astype(BF16)
    bp[:, O_TRIU:O_TRIU + 512] = fold_cols(np.triu(np.ones((T, T), f32))).astype(BF16)
    bp[:, O_IDB:O_IDB + 128] = np.eye(128, dtype=f32).astype(BF16)

    in_maps = []
    for e in range(E):
        sl = slice(e * HS, (e + 1) * HS)
        fpe = fp.copy()
        fpe[:, O_EID] = float(e)
        bpe = bp.copy()
        bpe[:, O_WUP:O_WUP + 2048] = fold_cols(
            np.ascontiguousarray(w_up_s[sl, :].astype(f32, copy=False).T)).astype(BF16)
        bpe[:, O_WGATE:O_WGATE + 2048] = fold_cols(
            np.ascontiguousarray(w_gate_s[sl, :].astype(f32, copy=False).T)).astype(BF16)
        bpe[:, O_WDOWN:O_WDOWN + 2048] = fold_cols(
            np.ascontiguousarray(w_down_s[:, sl].astype(f32, copy=False).T)).astype(BF16)
        m = {
            "f32pack": fpe,
            "bfpack": bpe,
            "upT": np.ascontiguousarray(up[e].astype(f32, copy=False).T.astype(BF16)),
            "gateT": np.ascontiguousarray(gate[e].astype(f32, copy=False).T.astype(BF16)),
            "downT": np.ascontiguousarray(down[e].astype(f32, copy=False).T.astype(BF16)),
        }
        in_maps.append(m)
    return in_maps


_make_in_maps = _pack_inputs


def run_spmd(in_maps, **kwargs):
    from concourse.bass_utils import run_bass_kernel_spmd

    nc = _get_program()
    return run_bass_kernel_spmd(nc, in_maps, core_ids=list(range(8)), **kwargs)


def kernel(x, up, gate, down, router, w_up_s, w_gate_s, w_down_s):
    in_maps = _pack_inputs(
        np.asarray(x), np.asarray(up), np.asarray(gate), np.asarray(down),
        np.asarray(router), np.asarray(w_up_s), np.asarray(w_gate_s),
        np.asarray(w_down_s),
    )
    res = run_spmd(in_maps)
    acc = np.zeros((C, T), np.float32)
    for i in range(E):
        acc += res.results[i]["outT"]
    return np.ascontiguousarray(acc.T).reshape(B, T, C).astype(np.float32)


# revision 19
# speedup vs baseline: 1.1495x; 1.0968x over previous
"""Expert-parallel top-1 MoE (SwiGLU experts + shared expert) on 8 TRN2 NeuronCores.

Strategy (hardcoded for B=1, T=256, C=1024, H=2048, E=8):
  - Core e holds expert e's weights (host pre-transposed, bf16) plus a
    1/8 slice (along H) of the shared expert.
  - Every core computes router logits in fp32 (exact argmax), builds a
    token->slot permutation matrix for its own expert on-device, gathers
    its tokens with a matmul, runs the SwiGLU FFN on <=128 token slots in
    bf16 (fp32 accumulation), and scatters results back with a matmul,
    accumulating its shared-expert partial in the same PSUM banks.
  - Each core writes a disjoint-support partial of the full [C, T] output;
    the host sums the 8 partials and transposes back to [1, T, C].

Schedule notes:
  - All small inputs ship as two packed buffers (one fp32, one bf16) so the
    routing path lands in ~2 DMAs instead of ~10.
  - Expert weights stream as ~1MB chunks, alternating the two HWDGE rings
    (sync + scalar), ordered by FFN consumption (H-half 0 first, down last).
  - A short burst of dummy matmuls warms the PE clock (HAM) while DMA runs.
"""

import sys

if "/opt/trn_rl_repo" not in sys.path:
    sys.path.insert(0, "/opt/trn_rl_repo")

import ml_dtypes
import numpy as np

B, T, C, H, E = 1, 256, 1024, 2048, 8
HS = H // 8        # shared-expert hidden slice per core
CCAP = 128         # per-expert token capacity (binomial mean 32; 128 is >12 sigma)
BF16 = ml_dtypes.bfloat16

# f32 pack layout (per-partition free offsets)
O_XT32, O_ROUT, O_IOTA, O_EID = 0, 2048, 2112, 2240
F32LEN = 2241
# bf16 pack layout
O_XB, O_XTB, O_WUP, O_WGATE, O_WDOWN, O_TRIU, O_IDB = (
    0, 2048, 4096, 6144, 8192, 10240, 10752)
BFLEN = 10880

N_WARM = 35

_CACHE = {}


def _build_program():
    import concourse.tile as tile
    from concourse import bacc, mybir

    f32 = mybir.dt.float32
    bf16 = mybir.dt.bfloat16
    u32 = mybir.dt.uint32
    ALU = mybir.AluOpType
    ACT = mybir.ActivationFunctionType

    nc = bacc.Bacc("TRN2", target_bir_lowering=False, debug=False, num_devices=8)

    f32pack = nc.dram_tensor("f32pack", [128, F32LEN], f32, kind="ExternalInput").ap()
    bfpack = nc.dram_tensor("bfpack", [128, BFLEN], bf16, kind="ExternalInput").ap()
    upT = nc.dram_tensor("upT", [C, H], bf16, kind="ExternalInput").ap()
    gateT = nc.dram_tensor("gateT", [C, H], bf16, kind="ExternalInput").ap()
    downT = nc.dram_tensor("downT", [H, C], bf16, kind="ExternalInput").ap()
    outT = nc.dram_tensor("outT", [C, T], f32, kind="ExternalOutput").ap()

    upTv = upT.rearrange("(a p) h -> p a h", p=128)        # [128, 8, 2048]
    gateTv = gateT.rearrange("(a p) h -> p a h", p=128)
    downTv = downT.rearrange("(a p) c -> p a c", p=128)    # [128, 16, 1024]
    outTv = outT.rearrange("(a p) t -> p a t", p=128)      # [128, 8, 256]

    with tile.TileContext(nc) as tc:
        with (
            tc.tile_pool(name="consts", bufs=1) as consts,
            tc.tile_pool(name="wts", bufs=1) as wts,
            tc.tile_pool(name="tmp", bufs=2) as tmp,
        ):
            # ---- packed small inputs ----
            # In-flight DMAs share SDMA bandwidth round-robin regardless of
            # issue order, so enforce strict phases (fp -> bp -> up/gate ->
            # down) with tiny gating copies on the otherwise-idle GpSimd
            # engine: each copy pre-writes one element of the next phase's
            # target tile while reading from the previous phase's tile, which
            # makes the next DMA wait (WAW) for the previous phase to land.
            fp_sb = consts.tile([128, F32LEN], f32, tag="fp")
            nc.sync.dma_start(fp_sb[:], f32pack[:])
            # pre-load the ACT engine's Silu table first thing on its queue
            warm_sb = consts.tile([128, 256], bf16, tag="warm")
            nc.vector.memset(warm_sb[:], 0.0)
            warm_act = tmp.tile([128, 8], bf16, tag="warm_act")
            nc.scalar.activation(warm_act[:], warm_sb[:, 0:8], ACT.Silu)
            bp_sb = consts.tile([128, BFLEN], bf16, tag="bp")
            nc.scalar.dma_start(bp_sb[:], bfpack[:])

            # slice helpers into the packs
            def xT32s(k, tt):           # fp32 x^T tile [128, 128] (lhsT for logits)
                o = O_XT32 + k * 256 + tt * 128
                return fp_sb[:, o:o + 128]

            def routs(k):               # routerT [128, 8]
                o = O_ROUT + k * 8
                return fp_sb[:, o:o + 8]

            iota_s = fp_sb[:, O_IOTA:O_IOTA + CCAP]
            eid_s = fp_sb[:, O_EID:O_EID + 1]

            def xbs(tt, m):             # x bf16 [128(t), 128(c)]
                o = O_XB + tt * 1024 + m * 128
                return bp_sb[:, o:o + 128]

            def xTbs(k):                # x^T bf16 [128, 256]
                o = O_XTB + k * 256
                return bp_sb[:, o:o + 256]

            def wups(k, st):
                o = O_WUP + k * 256 + st * 128
                return bp_sb[:, o:o + 128]

            def wgates(k, st):
                o = O_WGATE + k * 256 + st * 128
                return bp_sb[:, o:o + 128]

            def wdowns(st, m):
                o = O_WDOWN + st * 1024 + m * 128
                return bp_sb[:, o:o + 128]

            def trius(kt, mt):
                o = O_TRIU + kt * 256 + mt * 128
                return bp_sb[:, o:o + 128]

            idb_s = bp_sb[:, O_IDB:O_IDB + 128]

            # ---- expert weight chunks ----
            # up/gate: [128, 8 K-tiles, H-half] 2MB (one FFN half-consumption
            # unit); down: [128, 4 K-tiles, C] 1MB. Gating (via tiny GpSimd
            # copies creating WAW deps): up/gate wait for fp (so the router
            # path lands first); down waits for up half 1. The scalar ring
            # carries only bp + gate (its last issue unblocks well before the
            # first FFN silu needs the ACT queue).
            uph = [wts.tile([128, 8, 1024], bf16, tag=f"uph{i}", name=f"uph{i}")
                   for i in range(2)]
            gath = [wts.tile([128, 8, 1024], bf16, tag=f"gath{i}", name=f"gath{i}")
                    for i in range(2)]
            downc = [wts.tile([128, 4, C], bf16, tag=f"doc{i}", name=f"doc{i}")
                     for i in range(4)]
            for hh in range(2):
                hsl = slice(hh * 1024, (hh + 1) * 1024)
                nc.gpsimd.tensor_copy(uph[hh][:, 0, 0:1], fp_sb[:, 0:1])
                nc.sync.dma_start(uph[hh][:], upTv[:, :, hsl])
                nc.gpsimd.tensor_copy(gath[hh][:, 0, 0:1], fp_sb[:, 0:1])
                nc.scalar.dma_start(gath[hh][:], gateTv[:, :, hsl])
            for q in range(4):
                nc.gpsimd.tensor_copy(downc[q][:, 0, 0:1], uph[1][:, 0, 0:1])
                nc.sync.dma_start(downc[q][:], downTv[:, q * 4:(q + 1) * 4, :])

            # ---- PE warmup: dummy matmuls while DMA streams ----
            with tc.tile_pool(name="psW", bufs=1, space="PSUM") as psW:
                w_ps = psW.tile([128, 128], f32, tag="w")
                for _ in range(N_WARM):
                    nc.tensor.matmul(
                        w_ps[:], lhsT=warm_sb[:, 0:128], rhs=warm_sb[:, 128:256],
                        start=True, stop=True,
                    )

            # ---- routing (fp32 logits) + shared expert on PE ----
            mask_sb = consts.tile([128, 2, 1], f32, tag="mask")
            maskb_sb = consts.tile([128, 2, 1], bf16, tag="maskb")
            hsT_sb = consts.tile([128, 2, T], bf16, tag="hsT")
            possel_sb = consts.tile([128, 2, 1], f32, tag="possel")
            with tc.tile_pool(name="psA", bufs=2, space="PSUM") as psA:
                for tt in range(2):
                    lg_ps = psA.tile([128, E], f32, tag="lg")
                    for k in range(8):
                        nc.tensor.matmul(
                            lg_ps[:], lhsT=xT32s(k, tt), rhs=routs(k),
                            start=(k == 0), stop=(k == 7),
                        )
                    lg_sb = tmp.tile([128, E], f32, tag="lg_sb")
                    nc.vector.tensor_copy(lg_sb[:], lg_ps[:])
                    mx = tmp.tile([128, 8], f32, tag="mx")
                    nc.vector.max(mx[:], lg_sb[:])
                    mi = tmp.tile([128, 8], u32, tag="mi")
                    nc.vector.max_index(mi[:], mx[:], lg_sb[:])
                    idxf = tmp.tile([128, 1], f32, tag="idxf")
                    nc.vector.tensor_copy(idxf[:], mi[:, 0:1])
                    nc.vector.tensor_tensor(
                        mask_sb[:, tt, :], idxf[:], eid_s, op=ALU.is_equal
                    )
                    nc.vector.tensor_copy(maskb_sb[:, tt, :], mask_sb[:, tt, :])

                def shared_half(st):
                    us_ps = psA.tile([128, T], f32, tag="us")
                    gs_ps = psA.tile([128, T], f32, tag="gs")
                    for k in range(8):
                        nc.tensor.matmul(
                            us_ps[:], lhsT=wups(k, st), rhs=xTbs(k),
                            start=(k == 0), stop=(k == 7),
                        )
                        nc.tensor.matmul(
                            gs_ps[:], lhsT=wgates(k, st), rhs=xTbs(k),
                            start=(k == 0), stop=(k == 7),
                        )
                    sils = tmp.tile([128, T], bf16, tag="sils")
                    nc.scalar.activation(sils[:], gs_ps[:], ACT.Silu)
                    nc.vector.tensor_tensor(
                        hsT_sb[:, st, :], sils[:], us_ps[:], op=ALU.mult
                    )

                shared_half(0)

                # positions via triu matmul: cumsum(mask)[t] - 1, unrouted -> -2
                for mt in range(2):
                    pos_ps = psA.tile([128, 1], f32, tag="lg")
                    for kt in range(2):
                        nc.tensor.matmul(
                            pos_ps[:], lhsT=trius(kt, mt), rhs=maskb_sb[:, kt, :],
                            start=(kt == 0), stop=(kt == 1),
                        )
                    pos1 = tmp.tile([128, 1], f32, tag="pos1")
                    nc.vector.tensor_scalar(
                        pos1[:], pos_ps[:], 1.0, None, op0=ALU.add
                    )
                    posm = tmp.tile([128, 1], f32, tag="posm")
                    nc.vector.tensor_tensor(
                        posm[:], pos1[:], mask_sb[:, mt, :], op=ALU.mult
                    )
                    nc.vector.tensor_scalar(
                        possel_sb[:, mt, :], posm[:], 2.0, None, op0=ALU.subtract
                    )

                shared_half(1)

            # ---- permutation matrices + gather ----
            permT_sb = consts.tile([128, 2, CCAP], bf16, tag="permT")
            perm_sb = consts.tile([128, 2 * 128], bf16, tag="perm")
            gx_sb = consts.tile([128, 8, CCAP], bf16, tag="gx")
            with tc.tile_pool(name="psB", bufs=2, space="PSUM") as psB:
                for tt in range(2):
                    nc.vector.tensor_scalar(
                        permT_sb[:, tt, :], iota_s, possel_sb[:, tt, :], None,
                        op0=ALU.is_equal,
                    )
                for tt in range(2):
                    pt_ps = psB.tile([128, 128], bf16, tag="pt")
                    nc.tensor.transpose(pt_ps[:], permT_sb[:, tt, :], idb_s)
                    nc.vector.tensor_copy(
                        perm_sb[:, tt * 128:(tt + 1) * 128], pt_ps[:]
                    )
                for m in range(8):
                    g_ps = psB.tile([128, CCAP], f32, tag="gps")
                    for tt in range(2):
                        nc.tensor.matmul(
                            g_ps[:], lhsT=xbs(tt, m), rhs=permT_sb[:, tt, :],
                            start=(tt == 0), stop=(tt == 1),
                        )
                    nc.vector.tensor_copy(gx_sb[:, m, :], g_ps[:])

            # ---- routed FFN: tokens stationary, weights streaming ----
            hT_sb = consts.tile([128, 16, CCAP], bf16, tag="hT")
            y_sb = consts.tile([128, C], bf16, tag="y")
            with tc.tile_pool(name="psy", bufs=1, space="PSUM") as psy:
                y_ps = psy.tile([128, C], f32, tag="yps")
                with (
                    tc.tile_pool(name="psu", bufs=1, space="PSUM") as psu,
                    tc.tile_pool(name="pst", bufs=2, space="PSUM") as pst,
                ):
                    for hh in range(2):
                        u_ps = psu.tile([128, 1024], f32, tag="u")
                        g_ps = psu.tile([128, 1024], f32, tag="g")
                        for cc in range(2):
                            dst = slice(cc * 512, (cc + 1) * 512)
                            for k in range(8):
                                wsl = slice(cc * 512, (cc + 1) * 512)
                                nc.tensor.matmul(
                                    u_ps[:, dst], lhsT=gx_sb[:, k, :],
                                    rhs=uph[hh][:, k, wsl],
                                    start=(k == 0), stop=(k == 7),
                                )
                                nc.tensor.matmul(
                                    g_ps[:, dst], lhsT=gx_sb[:, k, :],
                                    rhs=gath[hh][:, k, wsl],
                                    start=(k == 0), stop=(k == 7),
                                )
                            sil = tmp.tile([128, 512], bf16, tag="sil")
                            nc.scalar.activation(sil[:], g_ps[:, dst], ACT.Silu)
                            h_sb = tmp.tile([128, 512], bf16, tag="h")
                            nc.vector.tensor_tensor(
                                h_sb[:], sil[:], u_ps[:, dst], op=ALU.mult
                            )
                            for j4 in range(4):
                                t_ps = pst.tile([128, 128], bf16, tag="tr")
                                nc.tensor.transpose(
                                    t_ps[:], h_sb[:, j4 * 128:(j4 + 1) * 128], idb_s
                                )
                                nc.vector.tensor_copy(
                                    hT_sb[:, hh * 8 + cc * 4 + j4, :], t_ps[:]
                                )

                # ---- down (C-half groups) + fused scatter/shared-down/out ----
                with tc.tile_pool(name="pso", bufs=2, space="PSUM") as pso:
                    for ccc in range(2):
                        dst = slice(ccc * 512, (ccc + 1) * 512)
                        for jj in range(16):
                            nc.tensor.matmul(
                                y_ps[:, dst], lhsT=hT_sb[:, jj, :],
                                rhs=downc[jj // 4][:, jj % 4, dst],
                                start=(jj == 0), stop=(jj == 15),
                            )
                        nc.vector.tensor_copy(y_sb[:, dst], y_ps[:, dst])
                        for half in range(2):
                            o_sb = tmp.tile([128, 2 * T], f32, tag="o_sb")
                            for mm in range(2):
                                m = ccc * 4 + half * 2 + mm
                                o_ps = pso.tile([128, T], f32, tag="o")
                                nc.tensor.matmul(
                                    o_ps[:], lhsT=y_sb[:, m * 128:(m + 1) * 128],
                                    rhs=perm_sb[:], start=True, stop=False,
                                )
                                for st in range(2):
                                    nc.tensor.matmul(
                                        o_ps[:], lhsT=wdowns(st, m),
                                        rhs=hsT_sb[:, st, :],
                                        start=False, stop=(st == 1),
                                    )
                                nc.vector.tensor_copy(
                                    o_sb[:, mm * T:(mm + 1) * T], o_ps[:]
                                )
                            nc.sync.dma_start(
                                outTv[:, ccc * 4 + half * 2:ccc * 4 + half * 2 + 2, :],
                                o_sb[:].rearrange("p (a t) -> p a t", t=T),
                            )

    nc.compile()
    return nc


def _get_program():
    if "nc" not in _CACHE:
        _CACHE["nc"] = _build_program()
    return _CACHE["nc"]


def _pack_inputs(x, up, gate, down, router, w_up_s, w_gate_s, w_down_s):
    f32 = np.float32
    x2 = np.ascontiguousarray(x.reshape(T, C)).astype(f32, copy=False)
    xT = np.ascontiguousarray(x2.T)

    def fold_cols(a):
        # [R, F] with R = n*128 -> [128, n*F] grouping k-tiles along free dim
        n = a.shape[0] // 128
        return a.reshape(n, 128, a.shape[1]).transpose(1, 0, 2).reshape(128, -1)

    fp = np.zeros((128, F32LEN), f32)
    fp[:, O_XT32:O_XT32 + 2048] = fold_cols(xT)
    fp[:, O_ROUT:O_ROUT + 64] = fold_cols(
        np.ascontiguousarray(router.astype(f32, copy=False).T))
    fp[:, O_IOTA:O_IOTA + CCAP] = np.arange(CCAP, dtype=f32)[None, :]

    bp = np.zeros((128, BFLEN), BF16)
    bp[:, O_XB:O_XB + 2048] = fold_cols(x2).astype(BF16)
    bp[:, O_XTB:O_XTB + 2048] = fold_cols(xT).astype(BF16)
    bp[:, O_TRIU:O_TRIU + 512] = fold_cols(np.triu(np.ones((T, T), f32))).astype(BF16)
    bp[:, O_IDB:O_IDB + 128] = np.eye(128, dtype=f32).astype(BF16)

    in_maps = []
    for e in range(E):
        sl = slice(e * HS, (e + 1) * HS)
        fpe = fp.copy()
        fpe[:, O_EID] = float(e)
        bpe = bp.copy()
        bpe[:, O_WUP:O_WUP + 2048] = fold_cols(
            np.ascontiguousarray(w_up_s[sl, :].astype(f32, copy=False).T)).astype(BF16)
        bpe[:, O_WGATE:O_WGATE + 2048] = fold_cols(
            np.ascontiguousarray(w_gate_s[sl, :].astype(f32, copy=False).T)).astype(BF16)
        bpe[:, O_WDOWN:O_WDOWN + 2048] = fold_cols(
            np.ascontiguousarray(w_down_s[:, sl].astype(f32, copy=False).T)).astype(BF16)
        m = {
            "f32pack": fpe,
            "bfpack": bpe,
            "upT": np.ascontiguousarray(up[e].astype(f32, copy=False).T.astype(BF16)),
            "gateT": np.ascontiguousarray(gate[e].astype(f32, copy=False).T.astype(BF16)),
            "downT": np.ascontiguousarray(down[e].astype(f32, copy=False).T.astype(BF16)),
        }
        in_maps.append(m)
    return in_maps


_make_in_maps = _pack_inputs


def run_spmd(in_maps, **kwargs):
    from concourse.bass_utils import run_bass_kernel_spmd

    nc = _get_program()
    return run_bass_kernel_spmd(nc, in_maps, core_ids=list(range(8)), **kwargs)


def kernel(x, up, gate, down, router, w_up_s, w_gate_s, w_down_s):
    in_maps = _pack_inputs(
        np.asarray(x), np.asarray(up), np.asarray(gate), np.asarray(down),
        np.asarray(router), np.asarray(w_up_s), np.asarray(w_gate_s),
        np.asarray(w_down_s),
    )
    res = run_spmd(in_maps)
    acc = np.zeros((C, T), np.float32)
    for i in range(E):
        acc += res.results[i]["outT"]
    return np.ascontiguousarray(acc.T).reshape(B, T, C).astype(np.float32)
